# revision 1
# baseline (speedup 1.0000x reference)
"""AttentionPooler Trainium2 kernel.

8-core data-parallel over batch (4 batches/core). Single pass over the large
encoder_outputs tensor (converted to fp16 on the host — halves HBM traffic;
all on-chip matmuls run fp16 at 1 cycle/row) with the small weights
algebraically folded on the host:

  scores[s,j] = x[s,:] @ Ac            Ac = column-centered gamma*q~^T/8
                                       (column-centering applies the
                                        LayerNorm mean subtraction exactly)
  es'[s,j] = exp(r_s*scores + ln r_s)  = r_s * exp(r_s*scores)
                                       (rstd folded into the exp bias, so
                                        the U matmul consumes RAW x — no
                                        768-wide x*r multiply on DVE)
  U[j,:]   = sum_s es'[s,j] * [x[s,:], mu_s, 1/r_s]   (PSUM accumulated)
  pooled   = (U[:, :768] - c1) / l     c1 = sum es' mu, l = sum es' / r = sum es
  ctx_h    = pooled_h @ (gamma*Wv)_h   per-head [32,768]@[768,64]
  out      = ctx @ Wo + beta@Wv@Wo

All PE transposes are replaced by XBAR DMA transposes (fp16-only HW path):
x^T comes straight from DRAM, pooled^T from SBUF.
"""
import numpy as np

import concourse.bass as bass
import concourse.bacc as bacc
import concourse.tile as tile
from concourse import mybir
from concourse.bass_utils import run_bass_kernel_spmd

# ---- problem constants (hardcoded per harness contract) ----
B, S, DIM = 32, 4096, 768
H, NQ, DH = 12, 32, 64
INNER = H * DH          # 768
J = H * NQ              # 384
N_CORES = 8
B_LOC = B // N_CORES    # 4
CHUNK = 128
N_CHUNKS = S // CHUNK   # 32
ET = DIM // 128         # 6 e-tiles of the model dim
JT = J // 128           # 3 j-tiles
EPS = 1e-5

F32 = mybir.dt.float32
F16 = mybir.dt.float16
AF = mybir.ActivationFunctionType
ALU = mybir.AluOpType


def _steer_act_tables(arch: str):
    """Make the act-table-load pass serve Exp from the set that also holds
    Ln, so a kernel alternating Ln/Exp loads tables exactly once."""
    from concourse.hw_specs import get_activation_tables

    tables = get_activation_tables(arch)  # functools.cache -> shared dict
    keep = "natural_log_exp_and_others"
    if keep in tables:
        for name, funcs in tables.items():
            if name != keep:
                funcs.discard(AF.Exp)


def _build_program():
    nc = bacc.Bacc(
        "TRN2", target_bir_lowering=False, debug=False, num_devices=N_CORES
    )
    _steer_act_tables(nc.m.arch)
    x_d = nc.dram_tensor("x", [B_LOC, S, DIM], F16, kind="ExternalInput")
    ac_d = nc.dram_tensor("ac", [128, ET, J], F16, kind="ExternalInput")
    wv_d = nc.dram_tensor("wv", [128, ET, INNER], F16, kind="ExternalInput")
    wo_d = nc.dram_tensor("wo", [128, ET, DIM], F16, kind="ExternalInput")
    id_d = nc.dram_tensor("ident", [128, 128], F16, kind="ExternalInput")
    y_d = nc.dram_tensor("y", [B_LOC, ET, 128, NQ], F32, kind="ExternalOutput")

    with tile.TileContext(nc) as tc, \
         tc.tile_pool(name="const", bufs=1) as const, \
         tc.tile_pool(name="xin", bufs=5) as xin, \
         tc.tile_pool(name="work", bufs=12) as work, \
         tc.tile_pool(name="stat", bufs=8) as stat, \
         tc.tile_pool(name="epi", bufs=2) as epi, \
         tc.tile_pool(name="pu", bufs=1, space="PSUM") as pu, \
         tc.tile_pool(name="pt", bufs=3, space="PSUM") as pt:

        ac_sb = const.tile([128, ET, J], F16, tag="ac")
        # wv/wo are first needed ~70us in (first epilogue); issue their DMAs
        # a few chunks into batch 0 so the first x chunks aren't queued
        # behind the weights.
        wv_sb = const.tile([128, ET, INNER], F16, tag="wv")
        wo_sb = const.tile([128, ET, DIM], F16, tag="wo")
        eps_sb = const.tile([128, 1], F32, tag="eps")
        nc.vector.memset(eps_sb[:], EPS)
        id_sb = const.tile([128, 128], F16, tag="ident")
        nc.scalar.dma_start(id_sb[:], id_d[:])

        TOT = B_LOC * N_CHUNKS
        u_tiles = {}
        stage_state = {}
        ep_state = {}

        GRAN = 4                       # chunks per DMA block (512 seq rows)
        NB = TOT // GRAN               # 32 blocks/core, 8 per batch

        def stage_a(bi):
            """Block DMA issue: one straight load (SP ring) + one XBAR
            transpose (ACT ring) covering GRAN chunks = 512 seq rows."""
            b, c0 = divmod(bi * GRAN, N_CHUNKS)
            src = x_d[b, c0 * 128:(c0 + GRAN) * 128, :]
            x_t = xin.tile([128, GRAN, DIM], F16, tag="x", name=f"x_{bi}")
            if bi == 0:
                for k in range(GRAN):
                    nc.sync.dma_start(
                        x_t[:, k, :], x_d[b, k * 128:(k + 1) * 128, :]
                    )
                    if k < 2:
                        # ac in two halves behind chunks 0/1: scores(0,0)
                        # only need e-tiles 0-2 to start (region-level deps)
                        nc.sync.dma_start(
                            ac_sb[:, 3 * k:3 * (k + 1), :],
                            ac_d[:, 3 * k:3 * (k + 1), :],
                        )

            else:
                nc.sync.dma_start(
                    x_t[:], src.rearrange("(sub p) e -> p sub e", p=128)
                )
            if bi == 2:
                nc.sync.dma_start(wv_sb[:], wv_d[:])
            if bi == 3:
                nc.sync.dma_start(wo_sb[:], wo_d[:])
            stage_state[("d", bi)] = x_t

        def stage_st(bi):
            """LayerNorm row stats for all GRAN sub-chunks of block bi."""
            x_t = stage_state.pop(("d", bi))
            st = stat.tile([128, 2 * GRAN, 6], F32, tag="st", name=f"st_{bi}")
            xg = x_t[:].rearrange("p sub (n f) -> p (sub n) f", f=384)
            for g in range(2 * GRAN):
                nc.vector.bn_stats(st[:, g, :], xg[:, g, :])
            mv = stat.tile([128, GRAN, 2], F16, tag="mv", name=f"mv_{bi}")
            for k in range(GRAN):
                nc.vector.bn_aggr(mv[:, k, :], st[:, 2 * k:2 * k + 2, :])
            # r = (var+eps)^-1/2 = exp(-0.5*ln(var+eps)); Ln+Exp share an ACT
            # table set (Rsqrt activation is banned for accuracy).
            lnv = stat.tile([128, GRAN], F32, tag="lnv", name=f"lnv_{bi}")
            nc.scalar.activation(lnv[:], mv[:, :, 1], AF.Ln,
                                 bias=eps_sb[:], scale=1.0)
            r_t = stat.tile([128, GRAN], F32, tag="r", name=f"r_{bi}")
            nc.scalar.activation(r_t[:], lnv[:], AF.Exp, scale=-0.5)
            # exp bias ln(r) = -0.5*lnv folds the rstd scale into es itself
            nb = stat.tile([128, GRAN], F32, tag="nb", name=f"nb_{bi}")
            nc.vector.tensor_scalar_mul(nb[:], lnv[:], -0.5)
            # overwrite the var slot with 1/r: mv becomes [mu, 1/r] -- the
            # two trailing U columns (c1 = sum es' mu, l = sum es'/r = sum es)
            nc.scalar.activation(mv[:, :, 1], lnv[:], AF.Exp, scale=0.5)
            stage_state[bi] = (x_t, mv, r_t, nb)

        def stage_b(bi):
            """scores + exp + U accumulation for block bi (U trails scores
            by one sub-chunk so PE never waits on the exp ACT latency; the
            last sub-chunk's U spills into the next block, except at batch
            boundaries where the U banks must close for the epilogue)."""
            x_t, mv, r_t, nb = stage_state.pop(bi)
            stage_state[("u", bi)] = (x_t, mv)
            for k in range(GRAN):
                xT = work.tile([128, DIM], F16, tag="xT", name=f"xT_{bi}_{k}")
                for half in range(2):
                    tp = pt.tile([128, 384], F16, tag="tp",
                                 name=f"tp_{bi}_{k}_{half}")
                    for t in range(3):
                        et = half * 3 + t
                        nc.tensor.transpose(
                            tp[:, t * 128:(t + 1) * 128],
                            x_t[:, k, et * 128:(et + 1) * 128],
                            id_sb[:],
                        )
                    dst = xT[:, half * 384:(half + 1) * 384]
                    if half == 0:
                        nc.scalar.copy(dst, tp[:])
                    else:
                        nc.vector.tensor_copy(dst, tp[:])
                sc = pt.tile([128, J], F32, tag="tp", name=f"sc_{bi}_{k}")
                for et in range(ET):
                    nc.tensor.matmul(
                        sc[:],
                        xT[:, et * 128:(et + 1) * 128],
                        ac_sb[:, et, :],
                        start=(et == 0), stop=(et == ET - 1),
                    )
                es = work.tile([128, J], F16, tag="es", name=f"es_{bi}_{k}")
                nc.scalar.activation(es[:], sc[:], AF.Exp,
                                     bias=nb[:, k:k + 1], scale=r_t[:, k:k + 1])
                stage_state[("v", bi, k)] = es
                # U for the PREVIOUS block, interleaved between this block's
                # score matmuls: every es the PE consumes is a full block old,
                # so PE never waits on the exp ACT latency.
                if ("u", bi - 1) in stage_state:
                    u_acc(bi - 1, k)
            if ("u", bi - 1) in stage_state:
                stage_state.pop(("u", bi - 1))
            if (bi + 1) % NB_B == 0:   # last block of a batch: close U now
                for k in range(GRAN):
                    u_acc(bi, k)
                stage_state.pop(("u", bi))

        def u_acc(bi, k):
            """U accumulation for sub-chunk k of block bi."""
            gi = bi * GRAN + k
            b, c = divmod(gi, N_CHUNKS)
            x4, mv4 = stage_state[("u", bi)]
            es = stage_state.pop(("v", bi, k))
            if c == 0:
                u_tiles[b] = (
                    [pu.tile([128, 512], F32, tag=f"u{jt}", name=f"u{jt}_{b}")
                     for jt in range(JT)],
                    pu.tile([128, 512], F32, tag="uhiA", name=f"uhiA_{b}"),
                    pu.tile([128, 512], F32, tag="uhiB", name=f"uhiB_{b}"),
                )
            ulo, uhiA, uhiB = u_tiles[b]
            # start=True clears has_written for a whole PSUM bank, so in each
            # shared bank only the first-emitted matmul of chunk 0 carries
            # start=True; later first-writes land as overwrites on cleared
            # bits (start=False).
            last = (c == N_CHUNKS - 1)
            for jt in range(JT):
                nc.tensor.matmul(
                    ulo[jt][:],
                    es[:, jt * 128:(jt + 1) * 128], x4[:, k, 0:512],
                    start=(c == 0), stop=last, skip_group_check=True,
                )
            for jt in range(JT):
                dst = (uhiA[:, jt * 256:(jt + 1) * 256] if jt < 2
                       else uhiB[:, 0:256])
                nc.tensor.matmul(
                    dst,
                    es[:, jt * 128:(jt + 1) * 128], x4[:, k, 512:768],
                    start=(c == 0 and jt != 1), stop=last,
                    skip_group_check=True,
                )
            for jt in range(JT):
                nc.tensor.matmul(
                    uhiB[:, 256 + 2 * jt:258 + 2 * jt],
                    es[:, jt * 128:(jt + 1) * 128], mv4[:, k, :],
                    start=False, stop=last, skip_group_check=True,
                )

        def ep1(b):
            """pooled = (U - c1)/l evacuation (DVE/ACT only, frees U banks)."""
            ulo, uhiA, uhiB = u_tiles[b]
            p2 = epi.tile([128, JT, DIM], F16, tag="p2", name=f"p2_{b}")
            for jt in range(JT):
                rl = stat.tile([128, 1], F32, tag="rl", name=f"rl_{b}_{jt}")
                nc.vector.reciprocal(rl[:], uhiB[:, 257 + 2 * jt:258 + 2 * jt])
                cc = stat.tile([128, 1], F32, tag="cc", name=f"cc_{b}_{jt}")
                nc.scalar.copy(cc[:], uhiB[:, 256 + 2 * jt:257 + 2 * jt])
                # all on DVE: the ACT engine is es-exp-critical exactly
                # when epilogues run (batch boundaries)
                nc.vector.tensor_scalar(
                    out=p2[:, jt, 0:512], in0=ulo[jt][:],
                    scalar1=cc[:], scalar2=rl[:],
                    op0=ALU.subtract, op1=ALU.mult,
                )
                nc.vector.tensor_scalar(
                    out=p2[:, jt, 512:768],
                    in0=(uhiA[:, jt * 256:(jt + 1) * 256] if jt < 2
                         else uhiB[:, 0:256]),
                    scalar1=cc[:], scalar2=rl[:],
                    op0=ALU.subtract, op1=ALU.mult,
                )
            ep_state[b] = p2

        def ep2(b):
            """transpose pooled -> p2T[e_local, et, j]: XBAR mid-stream (free
            DMA, hides behind chunk work), PE for the last batch (the tail
            has nothing to hide a DMA round trip behind)."""
            p2 = ep_state.pop(b)
            p2T = epi.tile([128, ET, J], F16, tag="p2T", name=f"p2T_{b}")
            if b < B_LOC - 1:
                for jt in range(JT):
                    nc.sync.dma_start_transpose(
                        p2T[:, :, jt * 128:(jt + 1) * 128], p2[:, jt, :]
                    )
            else:
                for et in range(ET):
                    tp = pt.tile([128, 384], F16, tag="tp",
                                 name=f"ep_tp_{b}_{et}")
                    for jt in range(JT):
                        nc.tensor.transpose(
                            tp[:, jt * 128:(jt + 1) * 128],
                            p2[:, jt, et * 128:(et + 1) * 128],
                            id_sb[:],
                        )
                    if et % 2 == 0:
                        nc.scalar.copy(p2T[:, et, :], tp[:])
                    else:
                        nc.vector.tensor_copy(p2T[:, et, :], tp[:])
            ep_state[b] = p2T

        def ep3(b):
            """ctx = pooled_h @ Wv'_h (pre-transposed), then out = ctx @ Wo."""
            p2T = ep_state.pop(b)
            ctxT = epi.tile([128, ET, NQ], F16, tag="ctxT", name=f"ctxT_{b}")
            for g in range(3):
                # 4 heads per PSUM bank (2 partition halves x 2 col groups):
                # one wide evacuation copy instead of four tiny ones
                cp = pt.tile([128, 2, NQ], F32, tag="tp", name=f"cp_{b}_g{g}")
                for hh in range(4):
                    h = g * 4 + hh
                    dst = cp[(hh % 2) * 64:(hh % 2) * 64 + 64, hh // 2, :]
                    for et in range(ET):
                        # start=True pends-zero the 2KB bank only on the
                        # partitions this matmul writes, so EACH partition
                        # half needs one bank-clearing first write (hh 0/1)
                        nc.tensor.matmul(
                            dst,
                            wv_sb[:, et, h * 64:(h + 1) * 64],
                            p2T[:, et, h * NQ:(h + 1) * NQ],
                            start=(et == 0 and hh <= 1), stop=(et == ET - 1),
                            skip_group_check=True,
                        )
                if g % 2 == 0:
                    nc.scalar.copy(ctxT[:, 2 * g:2 * g + 2, :], cp[:])
                else:
                    nc.vector.tensor_copy(ctxT[:, 2 * g:2 * g + 2, :], cp[:])

            # out^T: ocT[d_local, dt, n] = sum_f Wo[f, d] ctx[f, n] -- 32-col
            # moving tiles, 6x fewer PE rows than the straight orientation;
            # the host un-transposes.
            oc = epi.tile([128, ET, NQ], F32, tag="oc", name=f"oc_{b}")
            for g in range(2):
                # 3 output d-tiles per PSUM bank, one wide evacuation copy
                po = pt.tile([128, 3, NQ], F32, tag="tp", name=f"po_{b}_{g}")
                for dd in range(3):
                    dt = g * 3 + dd
                    for g2 in range(ET):
                        nc.tensor.matmul(
                            po[:, dd, :],
                            wo_sb[:, g2, dt * 128:(dt + 1) * 128],
                            ctxT[:, g2, :],
                            start=(g2 == 0 and dd == 0), stop=(g2 == ET - 1),
                            skip_group_check=True,
                        )
                if g == 0:
                    nc.scalar.copy(oc[:, 0:3, :], po[:])
                else:
                    nc.vector.tensor_copy(oc[:, 3:6, :], po[:])
            nc.sync.dma_start(y_d[b], oc[:])

        NB_B = NB // B_LOC             # blocks per batch
        for bi in range(NB + 4):
            if bi < NB:
                stage_a(bi)
            if 1 <= bi < NB + 1:
                stage_st(bi - 1)
            if 2 <= bi < NB + 2:
                stage_b(bi - 2)
            # epilogue pieces trail each batch's last stage_b by 0/1/2
            # iterations so their serial chains hide behind the next batch's
            # chunk work.
            for b in range(B_LOC):
                fin = (b + 1) * NB_B + 1   # bi at which stage_b(b, last)
                if bi == fin:
                    ep1(b)
                elif bi == fin + 1:
                    ep2(b)
                elif bi == fin + 2:
                    ep3(b)

    nc.compile()
    return nc


_NC_CACHE = None


def _get_program():
    global _NC_CACHE
    if _NC_CACHE is None:
        _NC_CACHE = _build_program()
    return _NC_CACHE


def _fold_weights(queries, Wq, Wkv, Wo, gamma, beta):
    """Host-side algebraic folding of the small weights (all fp32 numpy)."""
    q = queries.astype(np.float64) @ Wq.astype(np.float64)       # [32, 768]
    qh = q.reshape(NQ, H, DH)
    Wk = Wkv[:, :INNER].astype(np.float64)
    Wv = Wkv[:, INNER:].astype(np.float64)
    Wk_h = Wk.reshape(DIM, H, DH)
    # q~[j=(h,n), e] with j head-major
    qt = np.einsum("nhd,ehd->hne", qh, Wk_h, optimize=True).reshape(J, DIM)
    A = (gamma.astype(np.float64)[:, None] * qt.T) / (DH ** 0.5)  # [768, 384]
    Ac = A - A.mean(axis=0, keepdims=True)
    Wvp = gamma.astype(np.float64)[:, None] * Wv                  # [768, 768]
    bvwo = (beta.astype(np.float64) @ Wv) @ Wo.astype(np.float64)  # [768]

    def tile6(m):  # [768, F] -> [128, 6, F] e-tile-major layout
        return np.ascontiguousarray(
            m.reshape(ET, 128, -1).transpose(1, 0, 2)
        ).astype(np.float16)

    return (
        tile6(Ac),
        tile6(Wvp),
        tile6(Wo.astype(np.float64)),
        bvwo.astype(np.float32),
    )


def kernel(encoder_outputs, queries, Wq, Wkv, Wo, ln_gamma, ln_beta):
    x = np.ascontiguousarray(
        np.asarray(encoder_outputs, dtype=np.float32).astype(np.float16)
    )
    queries = np.asarray(queries, dtype=np.float32)
    Wq = np.asarray(Wq, dtype=np.float32)
    Wkv = np.asarray(Wkv, dtype=np.float32)
    Wo_np = np.asarray(Wo, dtype=np.float32)
    gamma = np.asarray(ln_gamma, dtype=np.float32)
    beta = np.asarray(ln_beta, dtype=np.float32)

    ac_t, wv_t, wo_t, bvwo = _fold_weights(queries, Wq, Wkv, Wo_np, gamma, beta)

    nc = _get_program()
    in_maps = [
        {
            "x": x[c * B_LOC:(c + 1) * B_LOC],
            "ac": ac_t,
            "wv": wv_t,
            "wo": wo_t,
            "ident": np.eye(128, dtype=np.float16),
        }
        for c in range(N_CORES)
    ]
    res = run_bass_kernel_spmd(nc, in_maps, list(range(N_CORES)))
    y = np.concatenate([res.results[c]["y"] for c in range(N_CORES)], axis=0)
    y = y.reshape(B, 128, ET, NQ).transpose(0, 3, 2, 1).reshape(B, NQ, DIM)
    return np.ascontiguousarray(y + bvwo[None, None, :]).astype(np.float32)



# revision 2
# speedup vs baseline: 1.3469x; 1.3469x over previous
"""AttentionPooler Trainium2 kernel.

8-core data-parallel over batch (4 batches/core). Single pass over the large
encoder_outputs tensor with the small weights algebraically folded on the host:

  scores[s,j] = x[s,:] @ Ac            Ac = column-centered gamma*q~^T/8
                                       (column-centering applies the
                                        LayerNorm mean subtraction exactly)
  es'[s,j] = exp(r_s*scores + ln r_s)  = r_s * exp(r_s*scores)
  U[j,:]   = sum_s es'[s,j] * [x[s,:], mu_s, 1/r_s]   (PSUM accumulated)
  pooled   = (U[:, :768] - c1) / l     c1 = sum es' mu, l = sum es' / r
  ctx_h    = pooled_h @ (gamma*Wv)_h   per-head [32,768]@[768,64]
  out      = ctx @ Wo + beta@Wv@Wo

The scores matmul runs in fp8e4 DoubleRow mode (0.5 PE cycles/row, K=256 per
instruction): the host uploads x pre-transposed and DR-packed in e4m3 at scale
2^6, and Ac as an e4m3 pair (value + same-scale residual — e4m3 subnormals
reach 2^-9, so the residual of a ~0.7-std value is representable at the same
scale), which removes the weight-quantization error. The 2^-6 descale is
folded into the per-row exp scale r. The U accumulation stays fp16 (stationary
es fp16, moving x fp16) for accuracy.
"""
import numpy as np
import ml_dtypes

import concourse.bass as bass
import concourse.bacc as bacc
import concourse.tile as tile
from concourse import mybir
from concourse.bass_utils import run_bass_kernel_spmd

# ---- problem constants (hardcoded per harness contract) ----
B, S, DIM = 32, 4096, 768
H, NQ, DH = 12, 32, 64
INNER = H * DH          # 768
J = H * NQ              # 384
N_CORES = 8
B_LOC = B // N_CORES    # 4
CHUNK = 128
N_CHUNKS = S // CHUNK   # 32
ET = DIM // 128         # 6 e-tiles of the model dim
KB = 3                  # DoubleRow k-blocks of 256 over the model dim
JT = J // 128           # 3 j-tiles
JH = 2                  # j-halves per DR scores matmul (384 = 2*192)
EPS = 1e-5
ACLOG2 = 6              # Ac uploaded at scale 2^ACLOG2

F32 = mybir.dt.float32
F16 = mybir.dt.float16
F8 = mybir.dt.float8e4
AF = mybir.ActivationFunctionType
ALU = mybir.AluOpType
E4NP = ml_dtypes.float8_e4m3fn


def _steer_act_tables(arch: str):
    """Make the act-table-load pass serve Exp from the set that also holds
    Ln, so a kernel alternating Ln/Exp loads tables exactly once."""
    from concourse.hw_specs import get_activation_tables

    tables = get_activation_tables(arch)  # functools.cache -> shared dict
    keep = "natural_log_exp_and_others"
    if keep in tables:
        for name, funcs in tables.items():
            if name != keep:
                funcs.discard(AF.Exp)


def _build_program():
    nc = bacc.Bacc(
        "TRN2", target_bir_lowering=False, debug=False, num_devices=N_CORES
    )
    _steer_act_tables(nc.m.arch)
    x_d = nc.dram_tensor("x", [B_LOC, S, DIM], F16, kind="ExternalInput")
    # pre-transposed DR-packed x: [b, chunk, p, kb*t*s] with
    # e_logical = kb*256 + t*128 + p
    xt_d = nc.dram_tensor(
        "xt", [B_LOC, N_CHUNKS, 128, KB * 2 * 128], F8, kind="ExternalInput"
    )
    ac_d = nc.dram_tensor("ac", [128, KB, 2, J], F8, kind="ExternalInput")
    dac_d = nc.dram_tensor("dac", [128, KB, 2, J], F8, kind="ExternalInput")
    wv_d = nc.dram_tensor("wv", [128, ET, INNER], F16, kind="ExternalInput")
    wo_d = nc.dram_tensor("wo", [128, ET, DIM], F16, kind="ExternalInput")
    id_d = nc.dram_tensor("ident", [128, 128], F16, kind="ExternalInput")
    y_d = nc.dram_tensor("y", [B_LOC, ET, 128, NQ], F32, kind="ExternalOutput")

    with tile.TileContext(nc) as tc, \
         tc.tile_pool(name="const", bufs=1) as const, \
         tc.tile_pool(name="xin", bufs=5) as xin, \
         tc.tile_pool(name="work", bufs=12) as work, \
         tc.tile_pool(name="stat", bufs=8) as stat, \
         tc.tile_pool(name="epi", bufs=2) as epi, \
         tc.tile_pool(name="pu", bufs=1, space="PSUM") as pu, \
         tc.tile_pool(name="pt", bufs=3, space="PSUM") as pt:

        ac_sb = const.tile([128, KB, 2, J], F8, tag="ac")
        dac_sb = const.tile([128, KB, 2, J], F8, tag="dac")
        # wv/wo are first needed ~at the first epilogue; issue their DMAs
        # a few blocks in so the first x chunks aren't queued behind them.
        wv_sb = const.tile([128, ET, INNER], F16, tag="wv")
        wo_sb = const.tile([128, ET, DIM], F16, tag="wo")
        eps_sb = const.tile([128, 1], F32, tag="eps")
        nc.vector.memset(eps_sb[:], EPS)
        # bias for the exp that produces the scores descale: ln(2^-ACLOG2)
        nl2_sb = const.tile([128, 1], F32, tag="nl2")
        nc.vector.memset(nl2_sb[:], -ACLOG2 * float(np.log(2.0)))
        id_sb = const.tile([128, 128], F16, tag="ident")
        nc.scalar.dma_start(id_sb[:], id_d[:])

        TOT = B_LOC * N_CHUNKS
        u_tiles = {}
        stage_state = {}
        ep_state = {}

        GRAN = 4                       # chunks per DMA block (512 seq rows)
        NB = TOT // GRAN               # 32 blocks/core, 8 per batch

        def stage_a(bi):
            """Block DMA issue: straight x (fp16) + pre-transposed x (fp8)."""
            b, c0 = divmod(bi * GRAN, N_CHUNKS)
            x_t = xin.tile([128, GRAN, DIM], F16, tag="x", name=f"x_{bi}")
            xt_t = xin.tile([128, GRAN, KB, 2, 128], F8, tag="xt",
                            name=f"xt_{bi}")
            if bi == 0:
                # weights the first scores matmuls need, ahead of the x queue
                nc.sync.dma_start(ac_sb[:], ac_d[:])
                nc.sync.dma_start(dac_sb[:], dac_d[:])
                for k in range(GRAN):
                    nc.sync.dma_start(
                        xt_t[:, k], xt_d[b, k].rearrange(
                            "p (kb t s) -> p kb t s", kb=KB, t=2
                        )
                    )
                    nc.sync.dma_start(
                        x_t[:, k, :], x_d[b, k * 128:(k + 1) * 128, :]
                    )
            else:
                nc.sync.dma_start(
                    xt_t[:], xt_d[b, c0:c0 + GRAN].rearrange(
                        "c p (kb t s) -> p c kb t s", kb=KB, t=2
                    )
                )
                src = x_d[b, c0 * 128:(c0 + GRAN) * 128, :]
                nc.sync.dma_start(
                    x_t[:], src.rearrange("(sub p) e -> p sub e", p=128)
                )
            if bi == 2:
                nc.sync.dma_start(wv_sb[:], wv_d[:])
            if bi == 3:
                nc.sync.dma_start(wo_sb[:], wo_d[:])
            stage_state[("d", bi)] = (x_t, xt_t)

        def stage_st(bi):
            """LayerNorm row stats for all GRAN sub-chunks of block bi."""
            x_t, xt_t = stage_state.pop(("d", bi))
            st = stat.tile([128, 2 * GRAN, 6], F32, tag="st", name=f"st_{bi}")
            xg = x_t[:].rearrange("p sub (n f) -> p (sub n) f", f=384)
            for g in range(2 * GRAN):
                nc.vector.bn_stats(st[:, g, :], xg[:, g, :])
            mv = stat.tile([128, GRAN, 2], F16, tag="mv", name=f"mv_{bi}")
            for k in range(GRAN):
                nc.vector.bn_aggr(mv[:, k, :], st[:, 2 * k:2 * k + 2, :])
            # r = (var+eps)^-1/2 = exp(-0.5*ln(var+eps)); Ln+Exp share an ACT
            # table set (Rsqrt activation is banned for accuracy).
            lnv = stat.tile([128, GRAN], F32, tag="lnv", name=f"lnv_{bi}")
            nc.scalar.activation(lnv[:], mv[:, :, 1], AF.Ln,
                                 bias=eps_sb[:], scale=1.0)
            # exp scale r2 = r * 2^-ACLOG2 (scores arrive pre-scaled by 2^6)
            r2_t = stat.tile([128, GRAN], F32, tag="r", name=f"r_{bi}")
            nc.scalar.activation(r2_t[:], lnv[:], AF.Exp, scale=-0.5,
                                 bias=nl2_sb[:])
            # exp bias ln(r) = -0.5*lnv folds the rstd scale into es itself
            nb = stat.tile([128, GRAN], F32, tag="nb", name=f"nb_{bi}")
            nc.vector.tensor_scalar_mul(nb[:], lnv[:], -0.5)
            # overwrite the var slot with 1/r: mv becomes [mu, 1/r] -- the
            # two trailing U columns (c1 = sum es' mu, l = sum es'/r = sum es)
            nc.scalar.activation(mv[:, :, 1], lnv[:], AF.Exp, scale=0.5)
            stage_state[bi] = (x_t, xt_t, mv, r2_t, nb)

        def stage_b(bi):
            """scores + exp + U accumulation for block bi (U trails scores
            by one block so PE never waits on the exp ACT latency; the
            last block's U spills into the next block, except at batch
            boundaries where the U banks must close for the epilogue)."""
            x_t, xt_t, mv, r2_t, nb = stage_state.pop(bi)
            stage_state[("u", bi)] = (x_t, mv)
            for k in range(GRAN):
                sc = pt.tile([128, J], F32, tag="tp", name=f"sc_{bi}_{k}")
                first = True
                for kb in range(KB):
                    for w8 in (ac_sb, dac_sb):
                        for jh in range(JH):
                            nc.tensor.matmul(
                                sc[:, jh * 192:(jh + 1) * 192],
                                xt_t[:, k, kb, :, :],
                                w8[:, kb, :, jh * 192:(jh + 1) * 192],
                                start=first,
                                stop=(kb == KB - 1 and w8 is dac_sb
                                      and jh == JH - 1),
                                perf_mode=mybir.MatmulPerfMode.DoubleRow,
                                skip_group_check=True,
                            )
                            first = False
                es = work.tile([128, J], F16, tag="es", name=f"es_{bi}_{k}")
                nc.scalar.activation(es[:], sc[:], AF.Exp,
                                     bias=nb[:, k:k + 1],
                                     scale=r2_t[:, k:k + 1])
                stage_state[("v", bi, k)] = es
                # U for the PREVIOUS block, interleaved between this block's
                # score matmuls: every es the PE consumes is a full block old,
                # so PE never waits on the exp ACT latency.
                if ("u", bi - 1) in stage_state:
                    u_acc(bi - 1, k)
            if ("u", bi - 1) in stage_state:
                stage_state.pop(("u", bi - 1))
            if (bi + 1) % NB_B == 0:   # last block of a batch: close U now
                for k in range(GRAN):
                    u_acc(bi, k)
                stage_state.pop(("u", bi))

        def u_acc(bi, k):
            """U accumulation for sub-chunk k of block bi."""
            gi = bi * GRAN + k
            b, c = divmod(gi, N_CHUNKS)
            x4, mv4 = stage_state[("u", bi)]
            es = stage_state.pop(("v", bi, k))
            if c == 0:
                u_tiles[b] = (
                    [pu.tile([128, 512], F32, tag=f"u{jt}", name=f"u{jt}_{b}")
                     for jt in range(JT)],
                    pu.tile([128, 512], F32, tag="uhiA", name=f"uhiA_{b}"),
                    pu.tile([128, 512], F32, tag="uhiB", name=f"uhiB_{b}"),
                )
            ulo, uhiA, uhiB = u_tiles[b]
            # start=True clears has_written for a whole PSUM bank, so in each
            # shared bank only the first-emitted matmul of chunk 0 carries
            # start=True; later first-writes land as overwrites on cleared
            # bits (start=False).
            last = (c == N_CHUNKS - 1)
            for jt in range(JT):
                nc.tensor.matmul(
                    ulo[jt][:],
                    es[:, jt * 128:(jt + 1) * 128], x4[:, k, 0:512],
                    start=(c == 0), stop=last, skip_group_check=True,
                )
            for jt in range(JT):
                dst = (uhiA[:, jt * 256:(jt + 1) * 256] if jt < 2
                       else uhiB[:, 0:256])
                nc.tensor.matmul(
                    dst,
                    es[:, jt * 128:(jt + 1) * 128], x4[:, k, 512:768],
                    start=(c == 0 and jt != 1), stop=last,
                    skip_group_check=True,
                )
            for jt in range(JT):
                nc.tensor.matmul(
                    uhiB[:, 256 + 2 * jt:258 + 2 * jt],
                    es[:, jt * 128:(jt + 1) * 128], mv4[:, k, :],
                    start=False, stop=last, skip_group_check=True,
                )

        def ep1(b):
            """pooled = (U - c1)/l evacuation (DVE/ACT only, frees U banks)."""
            ulo, uhiA, uhiB = u_tiles[b]
            p2 = epi.tile([128, JT, DIM], F16, tag="p2", name=f"p2_{b}")
            for jt in range(JT):
                rl = stat.tile([128, 1], F32, tag="rl", name=f"rl_{b}_{jt}")
                nc.vector.reciprocal(rl[:], uhiB[:, 257 + 2 * jt:258 + 2 * jt])
                cc = stat.tile([128, 1], F32, tag="cc", name=f"cc_{b}_{jt}")
                nc.scalar.copy(cc[:], uhiB[:, 256 + 2 * jt:257 + 2 * jt])
                # all on DVE: the ACT engine is es-exp-critical exactly
                # when epilogues run (batch boundaries)
                nc.vector.tensor_scalar(
                    out=p2[:, jt, 0:512], in0=ulo[jt][:],
                    scalar1=cc[:], scalar2=rl[:],
                    op0=ALU.subtract, op1=ALU.mult,
                )
                nc.vector.tensor_scalar(
                    out=p2[:, jt, 512:768],
                    in0=(uhiA[:, jt * 256:(jt + 1) * 256] if jt < 2
                         else uhiB[:, 0:256]),
                    scalar1=cc[:], scalar2=rl[:],
                    op0=ALU.subtract, op1=ALU.mult,
                )
            ep_state[b] = p2

        def ep2(b):
            """transpose pooled -> p2T[e_local, et, j]: XBAR mid-stream (free
            DMA, hides behind chunk work), PE for the last batch (the tail
            has nothing to hide a DMA round trip behind)."""
            p2 = ep_state.pop(b)
            p2T = epi.tile([128, ET, J], F16, tag="p2T", name=f"p2T_{b}")
            if b < B_LOC - 1:
                for jt in range(JT):
                    nc.sync.dma_start_transpose(
                        p2T[:, :, jt * 128:(jt + 1) * 128], p2[:, jt, :]
                    )
            else:
                for et in range(ET):
                    tp = pt.tile([128, 384], F16, tag="tp",
                                 name=f"ep_tp_{b}_{et}")
                    for jt in range(JT):
                        nc.tensor.transpose(
                            tp[:, jt * 128:(jt + 1) * 128],
                            p2[:, jt, et * 128:(et + 1) * 128],
                            id_sb[:],
                        )
                    if et % 2 == 0:
                        nc.scalar.copy(p2T[:, et, :], tp[:])
                    else:
                        nc.vector.tensor_copy(p2T[:, et, :], tp[:])
            ep_state[b] = p2T

        def ep3(b):
            """ctx = pooled_h @ Wv'_h (pre-transposed), then out = ctx @ Wo."""
            p2T = ep_state.pop(b)
            ctxT = epi.tile([128, ET, NQ], F16, tag="ctxT", name=f"ctxT_{b}")
            for g in range(3):
                # 4 heads per PSUM bank (2 partition halves x 2 col groups):
                # one wide evacuation copy instead of four tiny ones
                cp = pt.tile([128, 2, NQ], F32, tag="tp", name=f"cp_{b}_g{g}")
                for hh in range(4):
                    h = g * 4 + hh
                    dst = cp[(hh % 2) * 64:(hh % 2) * 64 + 64, hh // 2, :]
                    for et in range(ET):
                        # start=True pends-zero the 2KB bank only on the
                        # partitions this matmul writes, so EACH partition
                        # half needs one bank-clearing first write (hh 0/1)
                        nc.tensor.matmul(
                            dst,
                            wv_sb[:, et, h * 64:(h + 1) * 64],
                            p2T[:, et, h * NQ:(h + 1) * NQ],
                            start=(et == 0 and hh <= 1), stop=(et == ET - 1),
                            skip_group_check=True,
                        )
                if g % 2 == 0:
                    nc.scalar.copy(ctxT[:, 2 * g:2 * g + 2, :], cp[:])
                else:
                    nc.vector.tensor_copy(ctxT[:, 2 * g:2 * g + 2, :], cp[:])

            # out^T: ocT[d_local, dt, n] = sum_f Wo[f, d] ctx[f, n] -- 32-col
            # moving tiles, 6x fewer PE rows than the straight orientation;
            # the host un-transposes.
            oc = epi.tile([128, ET, NQ], F32, tag="oc", name=f"oc_{b}")
            for g in range(2):
                # 3 output d-tiles per PSUM bank, one wide evacuation copy
                po = pt.tile([128, 3, NQ], F32, tag="tp", name=f"po_{b}_{g}")
                for dd in range(3):
                    dt = g * 3 + dd
                    for g2 in range(ET):
                        nc.tensor.matmul(
                            po[:, dd, :],
                            wo_sb[:, g2, dt * 128:(dt + 1) * 128],
                            ctxT[:, g2, :],
                            start=(g2 == 0 and dd == 0), stop=(g2 == ET - 1),
                            skip_group_check=True,
                        )
                if g == 0:
                    nc.scalar.copy(oc[:, 0:3, :], po[:])
                else:
                    nc.vector.tensor_copy(oc[:, 3:6, :], po[:])
            nc.sync.dma_start(y_d[b], oc[:])

        NB_B = NB // B_LOC             # blocks per batch
        for bi in range(NB + 4):
            if bi < NB:
                stage_a(bi)
            if 1 <= bi < NB + 1:
                stage_st(bi - 1)
            if 2 <= bi < NB + 2:
                stage_b(bi - 2)
            # epilogue pieces trail each batch's last stage_b by 0/1/2
            # iterations so their serial chains hide behind the next batch's
            # chunk work.
            for b in range(B_LOC):
                fin = (b + 1) * NB_B + 1   # bi at which stage_b(b, last)
                if bi == fin:
                    ep1(b)
                elif bi == fin + 1:
                    ep2(b)
                elif bi == fin + 2:
                    ep3(b)

    nc.compile()
    return nc


_NC_CACHE = None


def _get_program():
    global _NC_CACHE
    if _NC_CACHE is None:
        _NC_CACHE = _build_program()
    return _NC_CACHE


def _fold_weights(queries, Wq, Wkv, Wo, gamma, beta):
    """Host-side algebraic folding of the small weights (all fp32 numpy)."""
    q = queries.astype(np.float64) @ Wq.astype(np.float64)       # [32, 768]
    qh = q.reshape(NQ, H, DH)
    Wk = Wkv[:, :INNER].astype(np.float64)
    Wv = Wkv[:, INNER:].astype(np.float64)
    Wk_h = Wk.reshape(DIM, H, DH)
    # q~[j=(h,n), e] with j head-major
    qt = np.einsum("nhd,ehd->hne", qh, Wk_h, optimize=True).reshape(J, DIM)
    A = (gamma.astype(np.float64)[:, None] * qt.T) / (DH ** 0.5)  # [768, 384]
    Ac = A - A.mean(axis=0, keepdims=True)
    Wvp = gamma.astype(np.float64)[:, None] * Wv                  # [768, 768]
    bvwo = (beta.astype(np.float64) @ Wv) @ Wo.astype(np.float64)  # [768]

    # Ac in e4m3 at scale 2^ACLOG2, plus a same-scale e4m3 residual; packed
    # [p, kb, t, j] with e_logical = kb*256 + t*128 + p
    acs = Ac * float(2 ** ACLOG2)                                 # [768, J]
    ac8 = acs.astype(np.float32).astype(E4NP)
    dac = (acs - ac8.astype(np.float64)).astype(np.float32).astype(E4NP)

    def dr_pack(m8):  # [768, J] -> [128, KB, 2, J]
        return np.ascontiguousarray(
            m8.reshape(KB, 2, 128, J).transpose(2, 0, 1, 3)
        )

    def tile6(m):  # [768, F] -> [128, 6, F] e-tile-major layout
        return np.ascontiguousarray(
            m.reshape(ET, 128, -1).transpose(1, 0, 2)
        ).astype(np.float16)

    return (
        dr_pack(ac8),
        dr_pack(dac),
        tile6(Wvp),
        tile6(Wo.astype(np.float64)),
        bvwo.astype(np.float32),
    )


def kernel(encoder_outputs, queries, Wq, Wkv, Wo, ln_gamma, ln_beta):
    enc = np.asarray(encoder_outputs, dtype=np.float32)
    x = np.ascontiguousarray(enc.astype(np.float16))
    # pre-transposed DR-packed e4m3 copy: [b, chunk, p, kb*2*128]
    x8 = enc.astype(E4NP)
    xt = np.ascontiguousarray(
        x8.reshape(B, N_CHUNKS, 128, KB, 2, 128).transpose(0, 1, 5, 3, 4, 2)
    ).reshape(B, N_CHUNKS, 128, KB * 2 * 128)
    queries = np.asarray(queries, dtype=np.float32)
    Wq = np.asarray(Wq, dtype=np.float32)
    Wkv = np.asarray(Wkv, dtype=np.float32)
    Wo_np = np.asarray(Wo, dtype=np.float32)
    gamma = np.asarray(ln_gamma, dtype=np.float32)
    beta = np.asarray(ln_beta, dtype=np.float32)

    ac8, dac8, wv_t, wo_t, bvwo = _fold_weights(
        queries, Wq, Wkv, Wo_np, gamma, beta
    )

    nc = _get_program()
    in_maps = [
        {
            "x": x[c * B_LOC:(c + 1) * B_LOC],
            "xt": xt[c * B_LOC:(c + 1) * B_LOC],
            "ac": ac8,
            "dac": dac8,
            "wv": wv_t,
            "wo": wo_t,
            "ident": np.eye(128, dtype=np.float16),
        }
        for c in range(N_CORES)
    ]
    res = run_bass_kernel_spmd(nc, in_maps, list(range(N_CORES)))
    y = np.concatenate([res.results[c]["y"] for c in range(N_CORES)], axis=0)
    y = y.reshape(B, 128, ET, NQ).transpose(0, 3, 2, 1).reshape(B, NQ, DIM)
    return np.ascontiguousarray(y + bvwo[None, None, :]).astype(np.float32)


# revision 12
# speedup vs baseline: 1.6038x; 1.1907x over previous
"""AttentionPooler Trainium2 kernel, V2: fp8 DoubleRow for BOTH big matmuls.

Same algebra as V1 (see kernel.py). Differences:
  - scores: unchanged (xT8 e4m3 DR-packed @ (ac8 + dac8)), 1152 PE cyc/chunk.
  - U accumulation now runs in e4m3 DoubleRow over superchunks of 256 seq
    rows with full residual correction:
        U = es8^T@(x8 + r8) + des8^T@x8
    where x8 = e4m3(x), r8 = e4m3(x - x8) (both uploaded), es8 = e4m3(es'),
    des8 = e4m3(es' - es8) (computed on ACT/GpSimd). The dropped des8@r8
    term is O(0.13%). 1731 PE cyc/chunk vs 2310 in fp16.
  - LayerNorm stats run on the fp8 x8 (exact in fp32 accumulation; the
    0.04%-scale stat bias is negligible); mu and 1/r ride as e4m3 columns
    768:770 of the x8 moving tile.
"""
import numpy as np
import ml_dtypes

import concourse.bass as bass
import concourse.bacc as bacc
import concourse.tile as tile
from concourse import mybir
from concourse.bass_utils import run_bass_kernel_spmd

B, S, DIM = 32, 4096, 768
H, NQ, DH = 12, 32, 64
INNER = H * DH
J = H * NQ              # 384
N_CORES = 8
B_LOC = B // N_CORES    # 4
CHUNK = 128
N_CHUNKS = S // CHUNK   # 32
ET = DIM // 128         # 6
KB = 3                  # DR k-blocks of 256 over the model dim
JT = J // 128           # 3
JH = 2
EPS = 1e-5
ACLOG2 = 6
XW = DIM + 16           # x8 moving row pitch: 768 + [mu8, invr8] + pad so the
                        # DoubleRow pair stride stays 16-byte aligned

F32 = mybir.dt.float32
F16 = mybir.dt.float16
F8 = mybir.dt.float8e4
AF = mybir.ActivationFunctionType
ALU = mybir.AluOpType
DR = mybir.MatmulPerfMode.DoubleRow
E4NP = ml_dtypes.float8_e4m3fn


def _steer_act_tables(arch: str):
    from concourse.hw_specs import get_activation_tables

    tables = get_activation_tables(arch)
    keep = "natural_log_exp_and_others"
    if keep in tables:
        for name, funcs in tables.items():
            if name != keep:
                funcs.discard(AF.Exp)


def _build_program():
    nc = bacc.Bacc(
        "TRN2", target_bir_lowering=False, debug=False, num_devices=N_CORES
    )
    _steer_act_tables(nc.m.arch)
    x8_d = nc.dram_tensor("x8", [B_LOC, S, DIM], F8, kind="ExternalInput")
    r8_d = nc.dram_tensor("r8", [B_LOC, S, DIM], F8, kind="ExternalInput")
    xt_d = nc.dram_tensor(
        "xt", [B_LOC, N_CHUNKS, 128, KB * 2 * 128], F8, kind="ExternalInput"
    )
    ac_d = nc.dram_tensor("ac", [128, KB, 2, J], F8, kind="ExternalInput")
    dac_d = nc.dram_tensor("dac", [128, KB, 2, J], F8, kind="ExternalInput")
    wv_d = nc.dram_tensor("wv", [128, ET, INNER], F16, kind="ExternalInput")
    wo_d = nc.dram_tensor("wo", [128, ET, DIM], F16, kind="ExternalInput")
    id_d = nc.dram_tensor("ident", [128, 128], F16, kind="ExternalInput")
    y_d = nc.dram_tensor("y", [B_LOC, 128, ET, NQ], F32, kind="ExternalOutput")

    with tile.TileContext(nc) as tc, \
         tc.tile_pool(name="const", bufs=1) as const, \
         tc.tile_pool(name="xin", bufs=5) as xin, \
         tc.tile_pool(name="work", bufs=12) as work, \
         tc.tile_pool(name="e8p", bufs=6) as e8p, \
         tc.tile_pool(name="stat", bufs=8) as stat, \
         tc.tile_pool(name="epi", bufs=2) as epi, \
         tc.tile_pool(name="pu", bufs=1, space="PSUM") as pu, \
         tc.tile_pool(name="pt", bufs=3, space="PSUM") as pt:

        ac_sb = const.tile([128, KB, 2, J], F8, tag="ac")
        dac_sb = const.tile([128, KB, 2, J], F8, tag="dac")
        wv_sb = const.tile([128, ET, INNER], F16, tag="wv")
        wo_sb = const.tile([128, ET, DIM], F16, tag="wo")
        eps_sb = const.tile([128, 1], F32, tag="eps")
        nc.vector.memset(eps_sb[:], EPS)
        nl2_sb = const.tile([128, 1], F32, tag="nl2")
        nc.vector.memset(nl2_sb[:], -ACLOG2 * float(np.log(2.0)))
        id_sb = const.tile([128, 128], F16, tag="ident")

        TOT = B_LOC * N_CHUNKS
        u_tiles = {}
        stage_state = {}
        ep_state = {}

        GRAN = 4                       # chunks per DMA block
        MS = GRAN // 2                 # superchunks per block
        NB = TOT // GRAN               # 32 blocks/core

        def stage_a(bi):
            b, c0 = divmod(bi * GRAN, N_CHUNKS)
            x8_t = xin.tile([128, MS, 2, XW], F8, tag="x8", name=f"x8_{bi}")
            r8_t = xin.tile([128, MS, 2, DIM], F8, tag="r8", name=f"r8_{bi}")
            xt_t = xin.tile([128, GRAN, KB, 2, 128], F8, tag="xt",
                            name=f"xt_{bi}")
            if bi == 0:
                for k in range(GRAN):
                    m, t = divmod(k, 2)
                    nc.sync.dma_start(
                        xt_t[:, k], xt_d[b, k].rearrange(
                            "p (kb t s) -> p kb t s", kb=KB, t=2
                        )
                    )
                    src = x8_d[b, k * 128:(k + 1) * 128, :]
                    nc.sync.dma_start(x8_t[:, m, t, 0:DIM], src)
                    nc.sync.dma_start(
                        r8_t[:, m, t, :], r8_d[b, k * 128:(k + 1) * 128, :]
                    )
                    if k < KB:
                        nc.sync.dma_start(ac_sb[:, k], ac_d[:, k])
                        nc.sync.dma_start(dac_sb[:, k], dac_d[:, k])
            else:
                nc.sync.dma_start(
                    xt_t[:], xt_d[b, c0:c0 + GRAN].rearrange(
                        "c p (kb t s) -> p c kb t s", kb=KB, t=2
                    )
                )
                src = x8_d[b, c0 * 128:(c0 + GRAN) * 128, :]
                nc.sync.dma_start(
                    x8_t[:, :, :, 0:DIM],
                    src.rearrange("(m t p) e -> p m t e", p=128, t=2)
                )
                src = r8_d[b, c0 * 128:(c0 + GRAN) * 128, :]
                nc.sync.dma_start(
                    r8_t[:], src.rearrange("(m t p) e -> p m t e", p=128, t=2)
                )
            if bi == 5:
                nc.sync.dma_start(wv_sb[:], wv_d[:])
            if bi == 6:
                nc.sync.dma_start(wo_sb[:], wo_d[:])
            if bi == 4:
                nc.scalar.dma_start(id_sb[:], id_d[:])
            stage_state[("d", bi)] = (x8_t, r8_t, xt_t)

        def stage_st(bi):
            """LayerNorm row stats from the fp8 x8; mu8/invr8 land as e4m3
            columns 768:770 of the x8 moving tile."""
            x8_t, r8_t, xt_t = stage_state.pop(("d", bi))
            st = stat.tile([128, 2 * GRAN, 6], F32, tag="st", name=f"st_{bi}")

            def xg(g):
                m, t = divmod(g // 2, 2)
                return x8_t[:, m, t, (g % 2) * 384:(g % 2 + 1) * 384]

            mv = stat.tile([128, GRAN, 2], F16, tag="mv", name=f"mv_{bi}")
            lnv = stat.tile([128, GRAN], F32, tag="lnv", name=f"lnv_{bi}")
            r2_t = stat.tile([128, GRAN], F32, tag="r", name=f"r_{bi}")
            nb = stat.tile([128, GRAN], F32, tag="nb", name=f"nb_{bi}")
            spans = ([(k, k + 1) for k in range(GRAN)] if bi == 0
                     else [(0, GRAN)])
            for k0, k1 in spans:
                for g in range(2 * k0, 2 * k1):
                    nc.vector.bn_stats(st[:, g, :], xg(g))
                for k in range(k0, k1):
                    nc.vector.bn_aggr(mv[:, k, :], st[:, 2 * k:2 * k + 2, :])
                nc.scalar.activation(lnv[:, k0:k1], mv[:, k0:k1, 1], AF.Ln,
                                     bias=eps_sb[:], scale=1.0)
                nc.scalar.activation(r2_t[:, k0:k1], lnv[:, k0:k1], AF.Exp,
                                     scale=-0.5, bias=nl2_sb[:])
                nc.vector.tensor_scalar_mul(nb[:, k0:k1], lnv[:, k0:k1], -0.5)
                for k in range(k0, k1):
                    m, t = divmod(k, 2)
                    nc.scalar.activation(x8_t[:, m, t, DIM:DIM + 1],
                                         mv[:, k, 0:1], AF.Copy)
                    nc.scalar.activation(x8_t[:, m, t, DIM + 1:DIM + 2],
                                         lnv[:, k:k + 1], AF.Exp, scale=0.5)
            stage_state[bi] = (x8_t, r8_t, xt_t, r2_t, nb)

        def stage_b(bi):
            x8_t, r8_t, xt_t, r2_t, nb = stage_state.pop(bi)
            stage_state[("u", bi)] = (x8_t, r8_t)
            for k in range(GRAN):
                m, t = divmod(k, 2)
                sc = pt.tile([128, J], F32, tag="tp", name=f"sc_{bi}_{k}")
                first = True
                for kb in range(KB):
                    for w8 in (ac_sb, dac_sb):
                        for jh in range(JH):
                            nc.tensor.matmul(
                                sc[:, jh * 192:(jh + 1) * 192],
                                xt_t[:, k, kb, :, :],
                                w8[:, kb, :, jh * 192:(jh + 1) * 192],
                                start=first,
                                stop=(kb == KB - 1 and w8 is dac_sb
                                      and jh == JH - 1),
                                perf_mode=DR, skip_group_check=True,
                            )
                            first = False
                es = work.tile([128, J], F16, tag="es", name=f"es_{bi}_{k}")
                nc.scalar.activation(es[:], sc[:], AF.Exp,
                                     bias=nb[:, k:k + 1],
                                     scale=r2_t[:, k:k + 1])
                if t == 0:
                    es8 = e8p.tile([128, 2, J], F8, tag="es8",
                                   name=f"es8_{bi}_{m}")
                    des8 = e8p.tile([128, 2, J], F8, tag="des8",
                                    name=f"des8_{bi}_{m}")
                    stage_state[("e8", bi, m)] = (es8, des8)
                else:
                    es8, des8 = stage_state[("e8", bi, m)]
                # es8 = e4m3(es') on ACT; des8 = es' - es8 on GpSimd (Pool) --
                # both engines have slack, DVE is stats-bound.
                nc.scalar.activation(es8[:, t, :], es[:], AF.Copy)
                nc.gpsimd.tensor_tensor(out=des8[:, t, :], in0=es[:],
                                        in1=es8[:, t, :], op=ALU.subtract)
                if t == 1 and ("u", bi - 1) in stage_state:
                    u_acc(bi - 1, m)
            if ("u", bi - 1) in stage_state:
                stage_state.pop(("u", bi - 1))
            if (bi + 1) % NB_B == 0:
                for m in range(MS):
                    u_acc(bi, m)
                stage_state.pop(("u", bi))

        def u_acc(bi, m):
            """DR U accumulation for superchunk m (256 seq rows) of block bi:
            es8@x8 + es8@r8 + des8@x8 into the shared U PSUM banks."""
            cw = (bi * GRAN + 2 * m) % N_CHUNKS
            b = (bi * GRAN + 2 * m) // N_CHUNKS
            x8_t, r8_t = stage_state[("u", bi)]
            es8, des8 = stage_state.pop(("e8", bi, m))
            if cw == 0:
                u_tiles[b] = (
                    [pu.tile([128, 512], F32, tag=f"u{jt}", name=f"u{jt}_{b}")
                     for jt in range(JT)],
                    pu.tile([128, 512], F32, tag="uhiA", name=f"uhiA_{b}"),
                    pu.tile([128, 512], F32, tag="uhiB", name=f"uhiB_{b}"),
                )
            ulo, uhiA, uhiB = u_tiles[b]
            first = (cw == 0)
            last = (cw == N_CHUNKS - 2)

            def terms(jt):
                return ((es8[:, :, jt * 128:(jt + 1) * 128], x8_t),
                        (es8[:, :, jt * 128:(jt + 1) * 128], r8_t),
                        (des8[:, :, jt * 128:(jt + 1) * 128], x8_t))

            for jt in range(JT):
                for h in range(2):
                    for ti, (st_ap, mov) in enumerate(terms(jt)):
                        nc.tensor.matmul(
                            ulo[jt][:, h * 256:(h + 1) * 256],
                            st_ap, mov[:, m, :, h * 256:(h + 1) * 256],
                            start=(first and h == 0 and ti == 0), stop=last,
                            perf_mode=DR, skip_group_check=True,
                        )
            for jt in range(JT):
                dst = (uhiA[:, jt * 256:(jt + 1) * 256] if jt < 2
                       else uhiB[:, 0:256])
                for ti, (st_ap, mov) in enumerate(terms(jt)):
                    nc.tensor.matmul(
                        dst, st_ap, mov[:, m, :, 512:768],
                        start=(first and ti == 0 and jt != 1), stop=last,
                        perf_mode=DR, skip_group_check=True,
                    )
            for jt in range(JT):
                # mu/invr columns: es8 and des8 against x8's cols 768:770
                for st_ap in (es8[:, :, jt * 128:(jt + 1) * 128],
                              des8[:, :, jt * 128:(jt + 1) * 128]):
                    nc.tensor.matmul(
                        uhiB[:, 256 + 2 * jt:258 + 2 * jt],
                        st_ap, x8_t[:, m, :, DIM:DIM + 2],
                        start=False, stop=last,
                        perf_mode=DR, skip_group_check=True,
                    )

        def ep1(b):
            ulo, uhiA, uhiB = u_tiles[b]
            p2 = epi.tile([128, JT, DIM], F16, tag="p2", name=f"p2_{b}")
            for jt in range(JT):
                rl = stat.tile([128, 1], F32, tag="rl", name=f"rl_{b}_{jt}")
                nc.vector.reciprocal(rl[:], uhiB[:, 257 + 2 * jt:258 + 2 * jt])
                cc = stat.tile([128, 1], F32, tag="cc", name=f"cc_{b}_{jt}")
                nc.scalar.copy(cc[:], uhiB[:, 256 + 2 * jt:257 + 2 * jt])
                nbias = stat.tile([128, 1], F32, tag="nbias",
                                  name=f"nbias_{b}_{jt}")
                nc.vector.tensor_scalar(
                    out=nbias[:], in0=cc[:], scalar1=rl[:], scalar2=-1.0,
                    op0=ALU.mult, op1=ALU.mult,
                )
                lo_dst, lo_src = p2[:, jt, 0:512], ulo[jt][:]
                hi_dst = p2[:, jt, 512:768]
                hi_src = (uhiA[:, jt * 256:(jt + 1) * 256] if jt < 2
                          else uhiB[:, 0:256])
                if jt % 2 == 0:
                    act_pairs, dve_pairs = [(lo_dst, lo_src)], \
                        [(hi_dst, hi_src)]
                else:
                    act_pairs, dve_pairs = [(hi_dst, hi_src)], \
                        [(lo_dst, lo_src)]
                for dst, src in act_pairs:
                    nc.scalar.activation(dst, src, AF.Identity,
                                         bias=nbias[:], scale=rl[:])
                for dst, src in dve_pairs:
                    nc.vector.tensor_scalar(
                        out=dst, in0=src,
                        scalar1=cc[:], scalar2=rl[:],
                        op0=ALU.subtract, op1=ALU.mult,
                    )
            ep_state[b] = p2

        def ep2(b):
            p2 = ep_state.pop(b)
            p2T = epi.tile([128, ET, J], F16, tag="p2T", name=f"p2T_{b}")
            if b < B_LOC - 1:
                for jt in range(JT):
                    nc.sync.dma_start_transpose(
                        p2T[:, :, jt * 128:(jt + 1) * 128], p2[:, jt, :]
                    )
            else:
                for et in range(ET):
                    tp = pt.tile([128, 384], F16, tag="tp",
                                 name=f"ep_tp_{b}_{et}")
                    for jt in range(JT):
                        nc.tensor.transpose(
                            tp[:, jt * 128:(jt + 1) * 128],
                            p2[:, jt, et * 128:(et + 1) * 128],
                            id_sb[:],
                        )
                    if et % 2 == 0:
                        nc.scalar.copy(p2T[:, et, :], tp[:])
                    else:
                        nc.vector.tensor_copy(p2T[:, et, :], tp[:])
            ep_state[b] = p2T

        def ep3(b):
            p2T = ep_state.pop(b)
            ctxT = epi.tile([128, ET, NQ], F16, tag="ctxT", name=f"ctxT_{b}")
            for g in range(3):
                cp = pt.tile([128, 2, NQ], F32, tag="tp", name=f"cp_{b}_g{g}")
                for hh in range(4):
                    h = g * 4 + hh
                    dst = cp[(hh % 2) * 64:(hh % 2) * 64 + 64, hh // 2, :]
                    for et in range(ET):
                        nc.tensor.matmul(
                            dst,
                            wv_sb[:, et, h * 64:(h + 1) * 64],
                            p2T[:, et, h * NQ:(h + 1) * NQ],
                            start=(et == 0 and hh <= 1), stop=(et == ET - 1),
                            skip_group_check=True,
                        )
                if g % 2 == 0:
                    nc.scalar.copy(ctxT[:, 2 * g:2 * g + 2, :], cp[:])
                else:
                    nc.vector.tensor_copy(ctxT[:, 2 * g:2 * g + 2, :], cp[:])

            oc = epi.tile([128, ET, NQ], F32, tag="oc", name=f"oc_{b}")
            for g in range(2):
                po = pt.tile([128, 3, NQ], F32, tag="tp", name=f"po_{b}_{g}")
                for dd in range(3):
                    dt = g * 3 + dd
                    for g2 in range(ET):
                        nc.tensor.matmul(
                            po[:, dd, :],
                            wo_sb[:, g2, dt * 128:(dt + 1) * 128],
                            ctxT[:, g2, :],
                            start=(g2 == 0 and dd == 0), stop=(g2 == ET - 1),
                            skip_group_check=True,
                        )
                if g == 0:
                    nc.scalar.copy(oc[:, 0:3, :], po[:])
                else:
                    nc.vector.tensor_copy(oc[:, 3:6, :], po[:])
                nc.sync.dma_start(y_d[b, :, 3 * g:3 * (g + 1), :],
                                  oc[:, 3 * g:3 * g + 3, :])

        NB_B = NB // B_LOC
        for bi in range(NB + 4):
            if bi < NB:
                stage_a(bi)
            if 1 <= bi < NB + 1:
                stage_st(bi - 1)
            if 2 <= bi < NB + 2:
                stage_b(bi - 2)
            for b in range(B_LOC):
                fin = (b + 1) * NB_B + 1
                if bi == fin:
                    ep1(b)
                elif bi == fin + 1:
                    ep2(b)
                elif bi == fin + 2:
                    ep3(b)

    nc.compile()
    return nc


_NC_CACHE = None


def _get_program():
    global _NC_CACHE
    if _NC_CACHE is None:
        _NC_CACHE = _build_program()
    return _NC_CACHE


def _fold_weights(queries, Wq, Wkv, Wo, gamma, beta):
    q = queries.astype(np.float64) @ Wq.astype(np.float64)
    qh = q.reshape(NQ, H, DH)
    Wk = Wkv[:, :INNER].astype(np.float64)
    Wv = Wkv[:, INNER:].astype(np.float64)
    Wk_h = Wk.reshape(DIM, H, DH)
    qt = np.einsum("nhd,ehd->hne", qh, Wk_h, optimize=True).reshape(J, DIM)
    A = (gamma.astype(np.float64)[:, None] * qt.T) / (DH ** 0.5)
    Ac = A - A.mean(axis=0, keepdims=True)
    Wvp = gamma.astype(np.float64)[:, None] * Wv
    bvwo = (beta.astype(np.float64) @ Wv) @ Wo.astype(np.float64)

    acs = Ac * float(2 ** ACLOG2)
    ac8 = acs.astype(np.float32).astype(E4NP)
    dac = (acs - ac8.astype(np.float64)).astype(np.float32).astype(E4NP)

    def dr_pack(m8):
        return np.ascontiguousarray(
            m8.reshape(KB, 2, 128, J).transpose(2, 0, 1, 3)
        )

    def tile6(m):
        return np.ascontiguousarray(
            m.reshape(ET, 128, -1).transpose(1, 0, 2)
        ).astype(np.float16)

    return (
        dr_pack(ac8),
        dr_pack(dac),
        tile6(Wvp),
        tile6(Wo.astype(np.float64)),
        bvwo.astype(np.float32),
    )


def kernel(encoder_outputs, queries, Wq, Wkv, Wo, ln_gamma, ln_beta):
    enc = np.asarray(encoder_outputs, dtype=np.float32)
    x8 = enc.astype(E4NP)
    r8 = (enc - x8.astype(np.float32)).astype(E4NP)
    xt = np.ascontiguousarray(
        x8.reshape(B, N_CHUNKS, 128, KB, 2, 128).transpose(0, 1, 5, 3, 4, 2)
    ).reshape(B, N_CHUNKS, 128, KB * 2 * 128)
    queries = np.asarray(queries, dtype=np.float32)
    Wq = np.asarray(Wq, dtype=np.float32)
    Wkv = np.asarray(Wkv, dtype=np.float32)
    Wo_np = np.asarray(Wo, dtype=np.float32)
    gamma = np.asarray(ln_gamma, dtype=np.float32)
    beta = np.asarray(ln_beta, dtype=np.float32)

    ac8, dac8, wv_t, wo_t, bvwo = _fold_weights(
        queries, Wq, Wkv, Wo_np, gamma, beta
    )

    nc = _get_program()
    in_maps = [
        {
            "x8": x8[c * B_LOC:(c + 1) * B_LOC],
            "r8": r8[c * B_LOC:(c + 1) * B_LOC],
            "xt": xt[c * B_LOC:(c + 1) * B_LOC],
            "ac": ac8,
            "dac": dac8,
            "wv": wv_t,
            "wo": wo_t,
            "ident": np.eye(128, dtype=np.float16),
        }
        for c in range(N_CORES)
    ]
    res = run_bass_kernel_spmd(nc, in_maps, list(range(N_CORES)))
    y = np.concatenate([res.results[c]["y"] for c in range(N_CORES)], axis=0)
    y = y.reshape(B, 128, ET, NQ).transpose(0, 3, 2, 1).reshape(B, NQ, DIM)
    return np.ascontiguousarray(y + bvwo[None, None, :]).astype(np.float32)
